# revision 12
# baseline (speedup 1.0000x reference)
"""Bass/Tile TRN2 kernel for nn_CPAMDec (CPAM cross-attention decoder).

Sharding: data-parallel over batch - 8 samples, one per NeuronCore.
All parameters are replicated; each core computes its full sample.

Host-side (parameter-only) preprocessing:
  - eval-mode BatchNorm affines folded into the adjacent 1x1-conv weights
  - the two chained fx convs fused into a single 512x512 matrix Wc
  - adaptive-pool block-mean scale (1/n_s) folded into encoder weights,
    so the device pools with raw block SUMS (no scalar muls)
  - all tensors pre-laid partition-major so every DMA is 128 descriptors
    of large contiguous spans

Device-side algebra (same reassociation as before): both 512x512 convs
over hw=5184 are eliminated through the 50-token bottleneck:
  sim  = (Wc@x + bc)^T @ fy  =  x^T @ G + const,   G = Wc^T @ fy [512,50]
  out  = Wup@(att@fself) + bup + x  =  FW^T @ [att;1]^T + x,
         FW = [fself @ Wup^T ; bup] [51,512]
Everything streams in fp16 (weights too); exp values are bf16.

Softmax uses a global logit shift K=48 folded into const (no per-pixel
max pass): logits for this model peak at ~80 and every pixel's max
logit is > 1, so exp(logit-48) neither overflows fp32/bf16 nor lets a
row's sum underflow (margin > 40 in the exponent on both sides).

Per-tile pipeline (12 tiles of 432 pixels), engines balanced:
  PE:  sim psum = sum_kc G_kc^T x_kc ; 4+4 transposes ; out mms
  ACT: e = exp(sim + const - 48) psum->sbuf ; attT psum->sbuf evac
  DVE: row sums + recip + 4 normalize muls ; residual adds (kc 0-1)
  POOL(gpsimd): exp-transpose evac copy ; residual adds (kc 2-3)
"""

import sys

for _p in ("/opt/trn_rl_repo", "/root/.axon_site/_ro/trn_rl_repo"):
    if _p not in sys.path:
        sys.path.append(_p)

import ml_dtypes
import numpy as np

import concourse.bacc as bacc
import concourse.bass as bass
import concourse.mybir as mybir
import concourse.tile as tile
from concourse.bass_utils import run_bass_kernel_spmd

F32 = mybir.dt.float32
BF16 = mybir.dt.bfloat16
FP16 = mybir.dt.float16
AX = mybir.AxisListType
AF = mybir.ActivationFunctionType
ALU = mybir.AluOpType

B, C, H, W = 8, 512, 72, 72
HW = H * W            # 5184
KC, P = 4, 128        # channel chunks x partitions
NT, TW = 12, 432      # hw tiles: 12 x (6 rows of 72)
NSUB, SUB = 4, 108    # row-subblocks per tile for softmax
NCH, CW = 6, 864      # stream chunks (2 tiles each) for both x and y
NPOOL = 50            # 1 + 4 + 9 + 36
NYT, YTC = 41, 7      # yT pixel-tiles (41 x 128 pix, padded) in 6 chunks of 7
EPS = 1e-5
KSH = 48.0            # global softmax logit shift
S_OFF = (0, 1, 5, 14)
S_LEN = (1, 4, 9, 36)
S_N = (5184, 1296, 576, 144)   # pool block sizes (folded into enc weights)

_NC = None


def _emit(nc):
    xd = nc.dram_tensor("xd", [P, NCH, KC, CW], FP16, kind="ExternalInput")
    ytd = nc.dram_tensor("ytd", [P, NYT, C], FP16, kind="ExternalInput")
    md = nc.dram_tensor("md", [P, NYT, 36], FP16, kind="ExternalInput")
    ad = nc.dram_tensor("ad", [36, 14], FP16, kind="ExternalInput")
    wxd = nc.dram_tensor("wxd", [P, 4 * KC, C], FP16, kind="ExternalInput")
    wyd = nc.dram_tensor("wyd", [P, 4 * KC, C], FP16, kind="ExternalInput")
    wcd = nc.dram_tensor("wcd", [P, KC, C], FP16, kind="ExternalInput")
    wupd = nc.dram_tensor("wupd", [P, KC, C], FP16, kind="ExternalInput")
    bcd = nc.dram_tensor("bcd", [P, KC], FP16, kind="ExternalInput")
    bupd = nc.dram_tensor("bupd", [1, C], FP16, kind="ExternalInput")
    bexd = nc.dram_tensor("bexd", [4, C], FP16, kind="ExternalInput")
    beyd = nc.dram_tensor("beyd", [4, C], FP16, kind="ExternalInput")
    lxd = nc.dram_tensor("lxd", [S_LEN[3], 4, NPOOL], FP16,
                         kind="ExternalInput")
    lyd = nc.dram_tensor("lyd", [S_LEN[3], 4, NPOOL], FP16,
                         kind="ExternalInput")
    bxd = nc.dram_tensor("bxd", [NPOOL, 1], F32, kind="ExternalInput")
    byd = nc.dram_tensor("byd", [NPOOL, 1], F32, kind="ExternalInput")
    onesd = nc.dram_tensor("onesd", [1, NT * TW], FP16, kind="ExternalInput")
    onespd = nc.dram_tensor("onespd", [P, 48], FP16, kind="ExternalInput")
    idfd = nc.dram_tensor("idfd", [P, P], FP16, kind="ExternalInput")
    idbd = nc.dram_tensor("idbd", [P, P], BF16, kind="ExternalInput")
    outd = nc.dram_tensor("outd", [P, NT, KC, TW], FP16,
                          kind="ExternalOutput")

    with tile.TileContext(nc) as tc:
        _body(nc, tc, xd, ytd, md, ad, wxd, wyd, wcd, wupd, bcd, bupd,
              bexd, beyd, lxd, lyd, bxd, byd, onesd, onespd, idfd, idbd,
              outd)
    nc.compile()
    return nc


def _body(nc, tc, xd, ytd, md, ad, wxd, wyd, wcd, wupd, bcd, bupd,
          bexd, beyd, lxd, lyd, bxd, byd, onesd, onespd, idfd, idbd, outd):
    from contextlib import ExitStack
    ctx = ExitStack()
    with ctx:
        ctx.enter_context(nc.allow_low_precision(
            reason="fp16 pool sums validated end-to-end on host"))
        consts = ctx.enter_context(tc.tile_pool(name="consts", bufs=1))
        xresp = ctx.enter_context(tc.tile_pool(name="xresp", bufs=1))
        poolp = ctx.enter_context(tc.tile_pool(name="poolp", bufs=1))
        ystr = ctx.enter_context(tc.tile_pool(name="ystr", bufs=2))
        encp = ctx.enter_context(tc.tile_pool(name="encp", bufs=1))
        encsp = ctx.enter_context(tc.tile_pool(name="encsp", bufs=2))
        esbp = ctx.enter_context(tc.tile_pool(name="esbp", bufs=2))
        attp = ctx.enter_context(tc.tile_pool(name="attp", bufs=2))
        attsp = ctx.enter_context(tc.tile_pool(name="attsp", bufs=1))
        outp = ctx.enter_context(tc.tile_pool(name="outp", bufs=3))

        # ---- constant tiles ----
        ident_f = consts.tile([P, P], FP16, tag="idf")
        ident_b = consts.tile([P, P], BF16, tag="idb")
        onesp = consts.tile([P, 48], FP16, tag="onesp")
        wy_sb = consts.tile([P, 4 * KC, C], FP16, tag="wy")
        wx_sb = consts.tile([P, 4 * KC, C], FP16, tag="wx")
        wc_sb = consts.tile([P, KC, C], FP16, tag="wc")
        wup_sb = consts.tile([P, KC, C], FP16, tag="wup")
        lyt_sb = consts.tile([S_LEN[3], 4, NPOOL], FP16, tag="lyt")
        lxt_sb = consts.tile([S_LEN[3], 4, NPOOL], FP16, tag="lxt")
        bey_sb = consts.tile([P, C], FP16, tag="bey")
        bex_sb = consts.tile([P, C], FP16, tag="bex")
        by_sb = consts.tile([NPOOL, 1], F32, tag="by")
        bx_sb = consts.tile([NPOOL, 1], F32, tag="bx")
        bc_sb = consts.tile([P, KC], FP16, tag="bc")

        # ---- persistent buffers ----
        x_sb = xresp.tile([P, NCH, KC, CW], FP16)
        partx = poolp.tile([P, NCH, KC, 72], FP16, tag="partx")
        m_sb = consts.tile([P, NYT, 36], FP16, tag="m")
        a_sb = consts.tile([36, 14], FP16, tag="a")
        p6y_sb = poolp.tile([36, C], FP16, tag="p6y")
        ch_sb = poolp.tile([14, C], FP16, tag="ch")
        pooledx = poolp.tile([P, KC, NPOOL], FP16, tag="pooledx")
        pooledy = poolp.tile([P, KC, NPOOL], FP16, tag="pooledy")
        attT_store = attsp.tile([NPOOL + 1, NT, TW], FP16, tag="attT")

        fy_sb = encp.tile([P, KC, NPOOL], FP16, tag="fy")
        fself_sb = encp.tile([P, KC, NPOOL], FP16, tag="fself")
        g_sb = encp.tile([P, KC, NPOOL], FP16, tag="g")
        const_sb = encp.tile([NPOOL, 1], F32, tag="const")
        fw_sb = encp.tile([NPOOL + 1, C], FP16, tag="fw")
        fyT = encp.tile([NPOOL, C], FP16, tag="fyT")
        fselfT = encp.tile([NPOOL, C], FP16, tag="fselfT")

        # ================= DMA schedule (issue order = priority) ========
        # triggers spread across idle sequencers: yT+smalls on gpsimd,
        # x chunks + out tiles on sync, weights on scalar(ACT)
        nc.gpsimd.dma_start(out=ident_f, in_=idfd.ap())
        nc.gpsimd.dma_start(out=ident_b, in_=idbd.ap())
        nc.gpsimd.dma_start(out=onesp, in_=onespd.ap())
        nc.gpsimd.dma_start(out=m_sb, in_=md.ap())
        nc.gpsimd.dma_start(out=a_sb, in_=ad.ap())
        nc.gpsimd.dma_start(
            out=attT_store[NPOOL:NPOOL + 1, :, :].rearrange(
                "p a b -> p (a b)"),
            in_=onesd.ap())

        # yT stream: 6 chunks of 7 pixel-tiles; pooled on the PE below
        yts = []
        for c in range(NCH):
            yt = ystr.tile([P, YTC, C], FP16, tag="yt")
            sl = slice(c * YTC, min((c + 1) * YTC, NYT))
            nc.gpsimd.dma_start(out=yt[:, :sl.stop - sl.start, :],
                                in_=ytd.ap()[:, sl])
            yts.append(yt)
        nc.scalar.dma_start(out=wy_sb, in_=wyd.ap())
        nc.gpsimd.dma_start(out=lyt_sb, in_=lyd.ap())
        for s in range(4):
            nc.gpsimd.dma_start(out=bey_sb[32 * s:32 * s + 1, :],
                                in_=beyd.ap()[s:s + 1, :])
        nc.gpsimd.dma_start(out=by_sb, in_=byd.ap())
        nc.gpsimd.dma_start(out=bc_sb, in_=bcd.ap())
        nc.scalar.dma_start(out=wc_sb, in_=wcd.ap())

        # x stream (partials interleaved into the sim loop below)
        x_dmas = []
        for c in range(NCH):
            d = nc.sync.dma_start(out=x_sb[:, c], in_=xd.ap()[:, c])
            x_dmas.append(d)
        nc.scalar.dma_start(out=wx_sb, in_=wxd.ap())
        nc.gpsimd.dma_start(out=lxt_sb, in_=lxd.ap())
        for s in range(4):
            nc.gpsimd.dma_start(out=bex_sb[32 * s:32 * s + 1, :],
                                in_=bexd.ap()[s:s + 1, :])
        nc.gpsimd.dma_start(out=bx_sb, in_=bxd.ap())
        nc.scalar.dma_start(out=wup_sb, in_=wupd.ap())
        nc.scalar.dma_start(out=fw_sb[NPOOL:NPOOL + 1, :], in_=bupd.ap())

        # ================= helpers ======================================
        def finish_pool(part, pooled):
            # part: [P, 6ch, KC, 72=(2tile 6row 6blk)] raw 12-col sums.
            # Writes raw block sums straight into pooled (scales folded
            # into the encoder weights host-side).
            s3 = poolp.tile([P, 6, 3], FP16, tag="s3")
            s2 = poolp.tile([P, 6, 2], FP16, tag="s2")
            for kc in range(KC):
                p6 = pooled[:, kc, 14:50]
                nc.vector.reduce_sum(
                    p6,
                    part[:, :, kc, :].rearrange(
                        "p c (tl lh wb) -> p c wb (tl lh)", tl=2, wb=6),
                    axis=AX.X)
                nc.vector.reduce_sum(
                    s3, p6.rearrange("p (hh w3 wl) -> p hh w3 wl",
                                     w3=3, wl=2), axis=AX.X)
                nc.vector.reduce_sum(
                    pooled[:, kc, 5:14],
                    s3.rearrange("p (h3 hl) w3 -> p h3 w3 hl", hl=2),
                    axis=AX.X)
                nc.vector.reduce_sum(
                    s2, p6.rearrange("p (hh w2 wl) -> p hh w2 wl",
                                     w2=2, wl=3), axis=AX.X)
                nc.vector.reduce_sum(
                    pooled[:, kc, 1:5],
                    s2.rearrange("p (h2 hl) w2 -> p h2 w2 hl", hl=3),
                    axis=AX.X)
                nc.vector.reduce_sum(pooled[:, kc, 0:1], p6, axis=AX.X)

        def encoder(ps_e, ps_m, pooled, wh, be, lt, b, fT, nm):
            # fT = lin(relu(W_s' @ pooled_s + b_s)) + b   [50, C] fp16
            fp = ps_m.tile([NPOOL, C], F32, tag="linps", name=nm + "lp")
            for s in range(4):
                off, ln = S_OFF[s], S_LEN[s]
                ep = ps_e.tile([S_LEN[3], C], F32, tag="encps",
                               name=nm + "ep")
                for kc in range(KC):
                    nc.tensor.matmul(
                        ep[:ln, :], pooled[:, kc, off:off + ln],
                        wh[:, s * KC + kc, :],
                        start=(kc == 0), stop=False)
                nc.tensor.matmul(ep[:ln, :],
                                 onesp[32 * s:32 * s + 1, :ln],
                                 be[32 * s:32 * s + 1, :],
                                 start=False, stop=True,
                                 tile_position=(32 * s, 0))
                enc_s = encsp.tile([S_LEN[3], C], FP16, tag="enc_s",
                                   name=nm + "es")
                nc.scalar.activation(enc_s[:ln, :], ep[:ln, :], AF.Relu)
                nc.tensor.matmul(fp, lt[:ln, s, :], enc_s[:ln, :],
                                 start=(s == 0), stop=(s == 3))
            nc.vector.tensor_scalar_add(fT, fp, b)

        def transpose_to_chunks(ps_m, fT, f_sb, nm):
            # fT [50, C] fp16 -> f_sb [P, KC, 50] fp16
            tp = ps_m.tile([P, KC, NPOOL], FP16, tag="tp", name=nm + "tp")
            for mc in range(KC):
                nc.tensor.transpose(tp[:, mc, :],
                                    fT[:, mc * P:(mc + 1) * P],
                                    ident_f[:NPOOL, :NPOOL])
            nc.vector.tensor_copy(f_sb, tp)

        # ================= y path: PE pooling -> encoder -> fy -> G =====
        # p6 = M^T @ yT (41 accumulating matmuls; also warms up the PE),
        # then [p1;p2;p3] = A^T @ p6, then transpose tokens to channels.
        with tc.tile_pool(name="ps_yp", bufs=1, space="PSUM") as pyp:
            p6p = pyp.tile([36, C], F32, tag="p6p")
            for t in range(NYT):
                nc.tensor.matmul(p6p, m_sb[:, t, :], yts[t // YTC][:, t % YTC, :],
                                 start=(t == 0), stop=(t == NYT - 1))
            nc.scalar.activation(p6y_sb, p6p, AF.Copy)
            chp = pyp.tile([14, C], F32, tag="chp")
            nc.tensor.matmul(chp, a_sb, p6y_sb, start=True, stop=True)
            nc.scalar.activation(ch_sb, chp, AF.Copy)
            ptp = pyp.tile([P, KC, NPOOL], FP16, tag="ptp")
            for mc in range(KC):
                nc.tensor.transpose(ptp[:, mc, 0:14],
                                    ch_sb[:, mc * P:(mc + 1) * P],
                                    ident_f[:14, :14])
                nc.tensor.transpose(ptp[:, mc, 14:50],
                                    p6y_sb[:, mc * P:(mc + 1) * P],
                                    ident_f[:36, :36])
            nc.vector.tensor_copy(pooledy, ptp)
        with tc.tile_pool(name="ps_ye", bufs=2, space="PSUM") as pye, \
             tc.tile_pool(name="ps_ym", bufs=1, space="PSUM") as pym:
            encoder(pye, pym, pooledy, wy_sb, bey_sb, lyt_sb, by_sb,
                    fyT, "y")
            transpose_to_chunks(pym, fyT, fy_sb, "y")
            # G = Wc^T @ fy  [4xP, 50]
            for mc in range(KC):
                gp = pym.tile([P, NPOOL], F32, tag="gps")
                for kc in range(KC):
                    nc.tensor.matmul(
                        gp, wc_sb[:, kc, mc * P:(mc + 1) * P],
                        fy_sb[:, kc, :],
                        start=(kc == 0), stop=(kc == KC - 1))
                nc.vector.tensor_copy(g_sb[:, mc, :], gp)
            # const = fy^T @ bc - KSH  [50, 1]
            cp = pym.tile([NPOOL, 1], F32, tag="cps")
            for kc in range(KC):
                nc.tensor.matmul(cp, fy_sb[:, kc, :], bc_sb[:, kc:kc + 1],
                                 start=(kc == 0), stop=(kc == KC - 1))
            nc.vector.tensor_scalar_add(const_sb, cp, -KSH)

        # ================= sim loop: sim -> softmax -> attT =============
        def x_view(t, kc):
            return x_sb[:, t // 2, kc, (t % 2) * TW:(t % 2 + 1) * TW]

        with tc.tile_pool(name="ps_sp", bufs=2, space="PSUM") as psp, \
             tc.tile_pool(name="ps_rp", bufs=2, space="PSUM") as prp, \
             tc.tile_pool(name="ps_ap", bufs=2, space="PSUM") as pap:
            for t in range(NT):
                if t % 2 == 0:
                    c = t // 2
                    nc.vector.reduce_sum(
                        partx[:, c],
                        x_sb[:, c].rearrange("p k (g wl) -> p (k g) wl",
                                             wl=12),
                        axis=AX.X)
                sp = psp.tile([NPOOL, TW], F32, tag="sp")
                for kc in range(KC):
                    nc.tensor.matmul(sp, g_sb[:, kc, :], x_view(t, kc),
                                     start=(kc == 0), stop=(kc == KC - 1))
                # e = exp(sim + const - KSH), bf16
                e_sb = esbp.tile([NPOOL, TW], BF16, tag="e")
                nc.scalar.activation(e_sb, sp, AF.Exp, bias=const_sb,
                                     scale=1.0)
                rp = prp.tile([SUB, NSUB, NPOOL], BF16, tag="rp")
                for j in range(NSUB):
                    nc.tensor.transpose(rp[:, j, :],
                                        e_sb[:, j * SUB:(j + 1) * SUB],
                                        ident_b[:NPOOL, :NPOOL])
                att_e = attp.tile([SUB, NSUB, NPOOL], BF16, tag="att_e")
                nc.scalar.activation(att_e, rp, AF.Copy)
                sums = attp.tile([SUB, NSUB], F32, tag="sums")
                rec = attp.tile([SUB, NSUB], F32, tag="rec")
                att_n = attp.tile([SUB, NSUB, NPOOL], FP16, tag="att_n")
                nc.vector.reduce_sum(sums, att_e, axis=AX.X)
                nc.vector.reciprocal(rec, sums)
                for j in range(NSUB):
                    nc.gpsimd.tensor_scalar_mul(
                        att_n[:, j, :], att_e[:, j, :], rec[:, j:j + 1])
                ap_ = pap.tile([NPOOL, TW], FP16, tag="ap")
                for j in range(NSUB):
                    nc.tensor.transpose(ap_[:, j * SUB:(j + 1) * SUB],
                                        att_n[:, j, :],
                                        ident_f[:SUB, :SUB])
                nc.scalar.activation(attT_store[:NPOOL, t, :], ap_,
                                     AF.Copy)

        # ================= x path: pool -> encoder -> fself -> FW =======
        finish_pool(partx, pooledx)
        with tc.tile_pool(name="ps_xe", bufs=2, space="PSUM") as pxe, \
             tc.tile_pool(name="ps_xm", bufs=1, space="PSUM") as pxm:
            encoder(pxe, pxm, pooledx, wx_sb, bex_sb, lxt_sb, bx_sb,
                    fselfT, "x")
            transpose_to_chunks(pxm, fselfT, fself_sb, "x")
            # FW = fself @ Wup'^T  [50, C]
            fwp = pxm.tile([NPOOL, C], F32, tag="fwp")
            for kc in range(KC):
                nc.tensor.matmul(fwp, fself_sb[:, kc, :], wup_sb[:, kc, :],
                                 start=(kc == 0), stop=(kc == KC - 1))
            nc.vector.tensor_copy(fw_sb[:NPOOL, :], fwp)

        # ================= out loop: out = FW^T @ attT + x ==============
        with tc.tile_pool(name="ps_o", bufs=3, space="PSUM") as pso:
            for t in range(NT):
                out_t = outp.tile([P, KC, TW], FP16, tag="out")
                for kc in range(KC):
                    op_ = pso.tile([P, TW], F32, tag="ops")
                    nc.tensor.matmul(op_,
                                     fw_sb[:, kc * P:(kc + 1) * P],
                                     attT_store[:, t, :],
                                     start=True, stop=True)
                    nc.vector.tensor_tensor(out_t[:, kc, :], op_,
                                            x_view(t, kc), ALU.add)
                nc.sync.dma_start(out=outd.ap()[:, t], in_=out_t)


def _split_lin(lw):
    # lin weight [50,50]; lhsT rows j split by pool scale -> [36, 4, 50]
    lt = lw.T.astype(np.float32)  # [j, k]
    out = np.zeros((4, S_LEN[3], NPOOL), np.float32)
    for s in range(4):
        out[s, :S_LEN[s]] = lt[S_OFF[s]:S_OFF[s] + S_LEN[s]]
    return np.ascontiguousarray(out.transpose(1, 0, 2))


def _bn_fold(bn):
    g, bt, m, v = [a.astype(np.float64) for a in bn]
    a = g / np.sqrt(v + EPS)
    return a, bt.astype(np.float64) - a * m


def _to16(a):
    return np.ascontiguousarray(a).astype(np.float16)


def _prep(inputs):
    """Host-side fold + shard. Returns list of 8 per-core input maps."""
    f = {k: np.asarray(v) for k, v in inputs.items()}

    a1, b1 = _bn_fold(f["fx_bn"][0])
    a2, b2 = _bn_fold(f["fx_bn"][1])
    W1 = f["fx_w"][0].astype(np.float64)
    W2 = f["fx_w"][1].astype(np.float64)
    Wc = (a2[:, None] * W2) @ (a1[:, None] * W1)
    bc = a2 * (W2 @ b1) + b2

    aup, bup = _bn_fold(f["fup_bn"])
    Wup = aup[:, None] * f["fup_w"].astype(np.float64)

    def enc_fold(w, bn):
        # fold BN affine AND the pool block-mean 1/n_s into the weights
        wts, bs = [], []
        for s in range(4):
            a, b = _bn_fold(bn[s])
            ws = (a[:, None] * w[s].astype(np.float64)) / S_N[s]
            wts.append(ws.T.reshape(KC, P, C))
            bs.append(b)
        # [4s, KC, P, C] -> [P, 4s*KC, C]
        wt = np.stack(wts).transpose(2, 0, 1, 3).reshape(P, 4 * KC, C)
        return _to16(wt), _to16(np.stack(bs))

    wxt, bex = enc_fold(f["enc_x_w"], f["enc_x_bn"])
    wyt, bey = enc_fold(f["enc_y_w"], f["enc_y_bn"])

    # block-indicator for PE pooling of yT: M[pix, b6] (12x12 blocks),
    # and the p6 -> [p1;p2;p3] aggregation matrix A [36, 14]
    NP_PAD = NYT * P
    pix = np.arange(NP_PAD)
    r, cc = pix // W, pix % W
    Mfull = np.zeros((NP_PAD, 36), np.float16)
    valid = pix < HW
    b6 = (r // 12) * 6 + (cc // 12)
    Mfull[valid, b6[valid]] = 1.0
    A = np.zeros((36, 14), np.float16)
    hh, wb = np.arange(36) // 6, np.arange(36) % 6
    A[:, 0] = 1.0
    A[np.arange(36), 1 + (hh // 3) * 2 + (wb // 3)] = 1.0
    A[np.arange(36), 5 + (hh // 2) * 3 + (wb // 2)] = 1.0

    common = {
        "md": np.ascontiguousarray(
            Mfull.reshape(NYT, P, 36).transpose(1, 0, 2)),
        "ad": A,
        "wxd": wxt, "wyd": wyt, "bexd": bex, "beyd": bey,
        "wcd": _to16(Wc.reshape(KC, P, C).transpose(1, 0, 2)),
        "wupd": _to16(
            np.ascontiguousarray(Wup.T).reshape(KC, P, C).transpose(1, 0, 2)),
        "bcd": _to16(bc.reshape(KC, P).T),
        "bupd": _to16(bup.reshape(1, C)),
        "lxd": _to16(_split_lin(f["lin_x_w"])),
        "lyd": _to16(_split_lin(f["lin_y_w"])),
        "bxd": f["lin_x_b"].astype(np.float32).reshape(NPOOL, 1).copy(),
        "byd": f["lin_y_b"].astype(np.float32).reshape(NPOOL, 1).copy(),
        "onesd": np.ones((1, HW), np.float16),
        "onespd": np.ones((P, 48), np.float16),
        "idfd": np.eye(P, dtype=np.float16),
        "idbd": np.eye(P).astype(ml_dtypes.bfloat16),
    }

    def shard_stream(a):
        # [C, H, W] -> [P, NCH, KC, CW] fp16
        v = a.reshape(KC, P, HW).transpose(1, 0, 2)      # [P, KC, HW]
        v = v.reshape(P, KC, NCH, CW).transpose(0, 2, 1, 3)
        return _to16(v)

    def shard_yT(a):
        # [C, H, W] -> yT padded [P, NYT, C] fp16 (pix-major, partition p
        # holds pixels p, 128+p, ... of the padded 5248-pixel image)
        v = np.zeros((NYT * P, C), np.float16)
        v[:HW] = a.reshape(C, HW).T.astype(np.float16)
        return np.ascontiguousarray(v.reshape(NYT, P, C).transpose(1, 0, 2))

    in_maps = []
    for i in range(B):
        m = dict(common)
        m["xd"] = shard_stream(f["x"][i])
        m["ytd"] = shard_yT(f["y"][i])
        in_maps.append(m)
    return in_maps


def _get_nc():
    global _NC
    if _NC is None:
        nc = bacc.Bacc("TRN2", target_bir_lowering=False)
        _NC = _emit(nc)
    return _NC


def _run(inputs, trace=False):
    nc = _get_nc()
    in_maps = _prep(inputs)
    res = run_bass_kernel_spmd(nc, in_maps, core_ids=list(range(B)),
                               trace=trace)
    out = np.empty((B, C, H, W), np.float32)
    for i in range(B):
        o = res.results[i]["outd"]                      # [P, NT, KC, TW]
        o = o.transpose(2, 0, 1, 3).reshape(C, HW)      # [C, HW]
        out[i] = o.astype(np.float32).reshape(C, H, W)
    return out, res


def kernel(**inputs) -> np.ndarray:
    out, _ = _run(inputs, trace=False)
    return out


# revision 13
# speedup vs baseline: 1.1256x; 1.1256x over previous
"""Bass/Tile TRN2 kernel for nn_CPAMDec (CPAM cross-attention decoder).

Sharding: data-parallel over batch - 8 samples, one per NeuronCore.
All parameters are replicated; each core computes its full sample.

Host-side (parameter-only) preprocessing:
  - eval-mode BatchNorm affines folded into the adjacent 1x1-conv weights
  - the two chained fx convs fused into a single 512x512 matrix Wc
  - adaptive-pool block-mean scale (1/n_s) folded into encoder weights,
    so the device pools with raw block SUMS (no scalar muls)
  - all tensors pre-laid partition-major so every DMA is 128 descriptors
    of large contiguous spans

Device-side algebra (same reassociation as before): both 512x512 convs
over hw=5184 are eliminated through the 50-token bottleneck:
  sim  = (Wc@x + bc)^T @ fy  =  x^T @ G + const,   G = Wc^T @ fy [512,50]
  out  = Wup@(att@fself) + bup + x  =  FW^T @ [att;1]^T + x,
         FW = [fself @ Wup^T ; bup] [51,512]
Everything streams in fp16 (weights too); exp values are bf16.

Softmax uses a global logit shift K=48 folded into const (no per-pixel
max pass): logits for this model peak at ~80 and every pixel's max
logit is > 1, so exp(logit-48) neither overflows fp32/bf16 nor lets a
row's sum underflow (margin > 40 in the exponent on both sides).

Per-tile pipeline (12 tiles of 432 pixels), engines balanced:
  PE:  sim psum = sum_kc G_kc^T x_kc ; 4+4 transposes ; out mms
  ACT: e = exp(sim + const - 48) psum->sbuf ; attT psum->sbuf evac
  DVE: row sums + recip + 4 normalize muls ; residual adds (kc 0-1)
  POOL(gpsimd): exp-transpose evac copy ; residual adds (kc 2-3)
"""

import sys

for _p in ("/opt/trn_rl_repo", "/root/.axon_site/_ro/trn_rl_repo"):
    if _p not in sys.path:
        sys.path.append(_p)

import ml_dtypes
import numpy as np

import concourse.bacc as bacc
import concourse.bass as bass
import concourse.mybir as mybir
import concourse.tile as tile
from concourse.bass_utils import run_bass_kernel_spmd

F32 = mybir.dt.float32
BF16 = mybir.dt.bfloat16
FP16 = mybir.dt.float16
AX = mybir.AxisListType
AF = mybir.ActivationFunctionType
ALU = mybir.AluOpType

B, C, H, W = 8, 512, 72, 72
HW = H * W            # 5184
KC, P = 4, 128        # channel chunks x partitions
NT, TW = 12, 432      # hw tiles: 12 x (6 rows of 72)
NSUB, SUB = 4, 108    # row-subblocks per tile for softmax
NCH, CW = 6, 864      # stream chunks (2 tiles each) for both x and y
NPOOL = 50            # 1 + 4 + 9 + 36
NYT, YTC = 41, 7      # yT pixel-tiles (41 x 128 pix, padded) in 6 chunks of 7
EPS = 1e-5
KSH = 48.0            # global softmax logit shift
S_OFF = (0, 1, 5, 14)
S_LEN = (1, 4, 9, 36)
S_N = (5184, 1296, 576, 144)   # pool block sizes (folded into enc weights)

_NC = None


def _emit(nc):
    xd = nc.dram_tensor("xd", [P, NCH, KC, CW], FP16, kind="ExternalInput")
    ytd = nc.dram_tensor("ytd", [P, NYT, C], FP16, kind="ExternalInput")
    md = nc.dram_tensor("md", [P, NYT, 36], FP16, kind="ExternalInput")
    ad = nc.dram_tensor("ad", [36, 14], FP16, kind="ExternalInput")
    wxd = nc.dram_tensor("wxd", [P, 4 * KC, C], FP16, kind="ExternalInput")
    wyd = nc.dram_tensor("wyd", [P, 4 * KC, C], FP16, kind="ExternalInput")
    wcd = nc.dram_tensor("wcd", [P, KC, C], FP16, kind="ExternalInput")
    wupd = nc.dram_tensor("wupd", [P, KC, C], FP16, kind="ExternalInput")
    bcd = nc.dram_tensor("bcd", [P, KC], FP16, kind="ExternalInput")
    bupd = nc.dram_tensor("bupd", [1, C], FP16, kind="ExternalInput")
    bexd = nc.dram_tensor("bexd", [4, C], FP16, kind="ExternalInput")
    beyd = nc.dram_tensor("beyd", [4, C], FP16, kind="ExternalInput")
    lxd = nc.dram_tensor("lxd", [S_LEN[3], 4, NPOOL], FP16,
                         kind="ExternalInput")
    lyd = nc.dram_tensor("lyd", [S_LEN[3], 4, NPOOL], FP16,
                         kind="ExternalInput")
    bxd = nc.dram_tensor("bxd", [NPOOL, 1], F32, kind="ExternalInput")
    byd = nc.dram_tensor("byd", [NPOOL, 1], F32, kind="ExternalInput")
    onesd = nc.dram_tensor("onesd", [1, NT * TW], FP16, kind="ExternalInput")
    onespd = nc.dram_tensor("onespd", [P, 48], FP16, kind="ExternalInput")
    idfd = nc.dram_tensor("idfd", [P, P], FP16, kind="ExternalInput")
    idbd = nc.dram_tensor("idbd", [P, P], BF16, kind="ExternalInput")
    outd = nc.dram_tensor("outd", [P, NT, KC, TW], FP16,
                          kind="ExternalOutput")

    with tile.TileContext(nc) as tc:
        _body(nc, tc, xd, ytd, md, ad, wxd, wyd, wcd, wupd, bcd, bupd,
              bexd, beyd, lxd, lyd, bxd, byd, onesd, onespd, idfd, idbd,
              outd)
    nc.compile()
    return nc


def _body(nc, tc, xd, ytd, md, ad, wxd, wyd, wcd, wupd, bcd, bupd,
          bexd, beyd, lxd, lyd, bxd, byd, onesd, onespd, idfd, idbd, outd):
    from contextlib import ExitStack
    ctx = ExitStack()
    with ctx:
        ctx.enter_context(nc.allow_low_precision(
            reason="fp16 pool sums validated end-to-end on host"))
        consts = ctx.enter_context(tc.tile_pool(name="consts", bufs=1))
        xresp = ctx.enter_context(tc.tile_pool(name="xresp", bufs=1))
        poolp = ctx.enter_context(tc.tile_pool(name="poolp", bufs=1))
        ystr = ctx.enter_context(tc.tile_pool(name="ystr", bufs=2))
        encp = ctx.enter_context(tc.tile_pool(name="encp", bufs=1))
        encsp = ctx.enter_context(tc.tile_pool(name="encsp", bufs=2))
        esbp = ctx.enter_context(tc.tile_pool(name="esbp", bufs=2))
        attp = ctx.enter_context(tc.tile_pool(name="attp", bufs=2))
        attsp = ctx.enter_context(tc.tile_pool(name="attsp", bufs=1))
        outp = ctx.enter_context(tc.tile_pool(name="outp", bufs=3))

        # ---- constant tiles ----
        ident_f = consts.tile([P, P], FP16, tag="idf")
        ident_b = consts.tile([P, P], BF16, tag="idb")
        onesp = consts.tile([P, 48], FP16, tag="onesp")
        wy_sb = consts.tile([P, 4 * KC, C], FP16, tag="wy")
        wx_sb = consts.tile([P, 4 * KC, C], FP16, tag="wx")
        wc_sb = consts.tile([P, KC, C], FP16, tag="wc")
        wup_sb = consts.tile([P, KC, C], FP16, tag="wup")
        lyt_sb = consts.tile([S_LEN[3], 4, NPOOL], FP16, tag="lyt")
        lxt_sb = consts.tile([S_LEN[3], 4, NPOOL], FP16, tag="lxt")
        bey_sb = consts.tile([P, C], FP16, tag="bey")
        bex_sb = consts.tile([P, C], FP16, tag="bex")
        by_sb = consts.tile([NPOOL, 1], F32, tag="by")
        bx_sb = consts.tile([NPOOL, 1], F32, tag="bx")
        bc_sb = consts.tile([P, KC], FP16, tag="bc")

        # ---- persistent buffers ----
        x_sb = xresp.tile([P, NCH, KC, CW], FP16)
        partx = poolp.tile([P, NCH, KC, 72], FP16, tag="partx")
        m_sb = consts.tile([P, NYT, 36], FP16, tag="m")
        a_sb = consts.tile([36, 14], FP16, tag="a")
        p6y_sb = poolp.tile([36, C], FP16, tag="p6y")
        ch_sb = poolp.tile([14, C], FP16, tag="ch")
        pooledx = poolp.tile([P, KC, NPOOL], FP16, tag="pooledx")
        pooledy = poolp.tile([P, KC, NPOOL], FP16, tag="pooledy")
        attT_store = attsp.tile([NPOOL + 1, NT, TW], FP16, tag="attT")

        fy_sb = encp.tile([P, KC, NPOOL], FP16, tag="fy")
        fself_sb = encp.tile([P, KC, NPOOL], FP16, tag="fself")
        g_sb = encp.tile([P, KC, NPOOL], FP16, tag="g")
        const_sb = encp.tile([NPOOL, 1], F32, tag="const")
        fw_sb = encp.tile([NPOOL + 1, C], FP16, tag="fw")
        fyT = encp.tile([NPOOL, C], FP16, tag="fyT")
        fselfT = encp.tile([NPOOL, C], FP16, tag="fselfT")

        # ================= DMA schedule (issue order = priority) ========
        # triggers spread across idle sequencers: yT+smalls on gpsimd,
        # x chunks + out tiles on sync, weights on scalar(ACT)
        nc.gpsimd.dma_start(out=ident_f, in_=idfd.ap())
        nc.gpsimd.dma_start(out=ident_b, in_=idbd.ap())
        nc.gpsimd.dma_start(out=onesp, in_=onespd.ap())
        nc.gpsimd.dma_start(out=m_sb, in_=md.ap())
        nc.gpsimd.dma_start(out=a_sb, in_=ad.ap())
        nc.gpsimd.dma_start(
            out=attT_store[NPOOL:NPOOL + 1, :, :].rearrange(
                "p a b -> p (a b)"),
            in_=onesd.ap())

        # yT stream first: 6 chunks of 7 pixel-tiles; pooled on the PE.
        # All input triggers on sync in strict priority order — hw queue
        # service follows trigger order, so this IS the landing order.
        yts = []
        for c in range(NCH):
            yt = ystr.tile([P, YTC, C], FP16, tag="yt")
            sl = slice(c * YTC, min((c + 1) * YTC, NYT))
            nc.sync.dma_start(out=yt[:, :sl.stop - sl.start, :],
                              in_=ytd.ap()[:, sl])
            yts.append(yt)
        nc.sync.dma_start(out=wy_sb, in_=wyd.ap())
        nc.sync.dma_start(out=wc_sb, in_=wcd.ap())
        nc.sync.dma_start(out=lyt_sb, in_=lyd.ap())
        for s in range(4):
            nc.sync.dma_start(out=bey_sb[32 * s:32 * s + 1, :],
                              in_=beyd.ap()[s:s + 1, :])
        nc.sync.dma_start(out=by_sb, in_=byd.ap())
        nc.sync.dma_start(out=bc_sb, in_=bcd.ap())

        # x stream (partials interleaved into the sim loop below)
        x_dmas = []
        for c in range(NCH):
            d = nc.sync.dma_start(out=x_sb[:, c], in_=xd.ap()[:, c])
            x_dmas.append(d)
        nc.sync.dma_start(out=wx_sb, in_=wxd.ap())
        nc.sync.dma_start(out=lxt_sb, in_=lxd.ap())
        for s in range(4):
            nc.sync.dma_start(out=bex_sb[32 * s:32 * s + 1, :],
                              in_=bexd.ap()[s:s + 1, :])
        nc.sync.dma_start(out=bx_sb, in_=bxd.ap())
        nc.sync.dma_start(out=wup_sb, in_=wupd.ap())
        nc.sync.dma_start(out=fw_sb[NPOOL:NPOOL + 1, :], in_=bupd.ap())

        # ================= helpers ======================================
        def finish_pool(part, pooled):
            # part: [P, 6ch, KC, 72=(2tile 6row 6blk)] raw 12-col sums.
            # Writes raw block sums straight into pooled (scales folded
            # into the encoder weights host-side).
            s3 = poolp.tile([P, 6, 3], FP16, tag="s3")
            s2 = poolp.tile([P, 6, 2], FP16, tag="s2")
            for kc in range(KC):
                p6 = pooled[:, kc, 14:50]
                nc.vector.reduce_sum(
                    p6,
                    part[:, :, kc, :].rearrange(
                        "p c (tl lh wb) -> p c wb (tl lh)", tl=2, wb=6),
                    axis=AX.X)
                nc.vector.reduce_sum(
                    s3, p6.rearrange("p (hh w3 wl) -> p hh w3 wl",
                                     w3=3, wl=2), axis=AX.X)
                nc.vector.reduce_sum(
                    pooled[:, kc, 5:14],
                    s3.rearrange("p (h3 hl) w3 -> p h3 w3 hl", hl=2),
                    axis=AX.X)
                nc.vector.reduce_sum(
                    s2, p6.rearrange("p (hh w2 wl) -> p hh w2 wl",
                                     w2=2, wl=3), axis=AX.X)
                nc.vector.reduce_sum(
                    pooled[:, kc, 1:5],
                    s2.rearrange("p (h2 hl) w2 -> p h2 w2 hl", hl=3),
                    axis=AX.X)
                nc.vector.reduce_sum(pooled[:, kc, 0:1], p6, axis=AX.X)

        def encoder(ps_e, ps_m, pooled, wh, be, lt, b, fT, nm):
            # fT = lin(relu(W_s' @ pooled_s + b_s)) + b   [50, C] fp16
            fp = ps_m.tile([NPOOL, C], F32, tag="linps", name=nm + "lp")
            for s in range(4):
                off, ln = S_OFF[s], S_LEN[s]
                ep = ps_e.tile([S_LEN[3], C], F32, tag="encps",
                               name=nm + "ep")
                for kc in range(KC):
                    nc.tensor.matmul(
                        ep[:ln, :], pooled[:, kc, off:off + ln],
                        wh[:, s * KC + kc, :],
                        start=(kc == 0), stop=False)
                nc.tensor.matmul(ep[:ln, :],
                                 onesp[32 * s:32 * s + 1, :ln],
                                 be[32 * s:32 * s + 1, :],
                                 start=False, stop=True,
                                 tile_position=(32 * s, 0))
                enc_s = encsp.tile([S_LEN[3], C], FP16, tag="enc_s",
                                   name=nm + "es")
                nc.scalar.activation(enc_s[:ln, :], ep[:ln, :], AF.Relu)
                nc.tensor.matmul(fp, lt[:ln, s, :], enc_s[:ln, :],
                                 start=(s == 0), stop=(s == 3))
            nc.vector.tensor_scalar_add(fT, fp, b)

        def transpose_to_chunks(ps_m, fT, f_sb, nm):
            # fT [50, C] fp16 -> f_sb [P, KC, 50] fp16
            tp = ps_m.tile([P, KC, NPOOL], FP16, tag="tp", name=nm + "tp")
            for mc in range(KC):
                nc.tensor.transpose(tp[:, mc, :],
                                    fT[:, mc * P:(mc + 1) * P],
                                    ident_f[:NPOOL, :NPOOL])
            nc.vector.tensor_copy(f_sb, tp)

        # ================= y path: PE pooling -> encoder -> fy -> G =====
        # p6 = M^T @ yT (41 accumulating matmuls; also warms up the PE),
        # then [p1;p2;p3] = A^T @ p6, then transpose tokens to channels.
        with tc.tile_pool(name="ps_yp", bufs=1, space="PSUM") as pyp:
            p6p = pyp.tile([36, C], F32, tag="p6p")
            for t in range(NYT):
                nc.tensor.matmul(p6p, m_sb[:, t, :], yts[t // YTC][:, t % YTC, :],
                                 start=(t == 0), stop=(t == NYT - 1))
            nc.scalar.activation(p6y_sb, p6p, AF.Copy)
            chp = pyp.tile([14, C], F32, tag="chp")
            nc.tensor.matmul(chp, a_sb, p6y_sb, start=True, stop=True)
            nc.scalar.activation(ch_sb, chp, AF.Copy)
            ptp = pyp.tile([P, KC, NPOOL], FP16, tag="ptp")
            for mc in range(KC):
                nc.tensor.transpose(ptp[:, mc, 0:14],
                                    ch_sb[:, mc * P:(mc + 1) * P],
                                    ident_f[:14, :14])
                nc.tensor.transpose(ptp[:, mc, 14:50],
                                    p6y_sb[:, mc * P:(mc + 1) * P],
                                    ident_f[:36, :36])
            nc.vector.tensor_copy(pooledy, ptp)
        with tc.tile_pool(name="ps_ye", bufs=2, space="PSUM") as pye, \
             tc.tile_pool(name="ps_ym", bufs=1, space="PSUM") as pym:
            encoder(pye, pym, pooledy, wy_sb, bey_sb, lyt_sb, by_sb,
                    fyT, "y")
            transpose_to_chunks(pym, fyT, fy_sb, "y")
            # G = Wc^T @ fy  [4xP, 50]
            for mc in range(KC):
                gp = pym.tile([P, NPOOL], F32, tag="gps")
                for kc in range(KC):
                    nc.tensor.matmul(
                        gp, wc_sb[:, kc, mc * P:(mc + 1) * P],
                        fy_sb[:, kc, :],
                        start=(kc == 0), stop=(kc == KC - 1))
                nc.vector.tensor_copy(g_sb[:, mc, :], gp)
            # const = fy^T @ bc - KSH  [50, 1]
            cp = pym.tile([NPOOL, 1], F32, tag="cps")
            for kc in range(KC):
                nc.tensor.matmul(cp, fy_sb[:, kc, :], bc_sb[:, kc:kc + 1],
                                 start=(kc == 0), stop=(kc == KC - 1))
            nc.vector.tensor_scalar_add(const_sb, cp, -KSH)

        # ================= sim loop: sim -> softmax -> attT =============
        def x_view(t, kc):
            return x_sb[:, t // 2, kc, (t % 2) * TW:(t % 2 + 1) * TW]

        with tc.tile_pool(name="ps_sp", bufs=2, space="PSUM") as psp, \
             tc.tile_pool(name="ps_rp", bufs=2, space="PSUM") as prp, \
             tc.tile_pool(name="ps_ap", bufs=2, space="PSUM") as pap:
            for t in range(NT):
                if t % 2 == 0:
                    c = t // 2
                    nc.vector.reduce_sum(
                        partx[:, c],
                        x_sb[:, c].rearrange("p k (g wl) -> p (k g) wl",
                                             wl=12),
                        axis=AX.X)
                sp = psp.tile([NPOOL, TW], F32, tag="sp")
                for kc in range(KC):
                    nc.tensor.matmul(sp, g_sb[:, kc, :], x_view(t, kc),
                                     start=(kc == 0), stop=(kc == KC - 1))
                # e = exp(sim + const - KSH), bf16
                e_sb = esbp.tile([NPOOL, TW], BF16, tag="e")
                nc.scalar.activation(e_sb, sp, AF.Exp, bias=const_sb,
                                     scale=1.0)
                rp = prp.tile([SUB, NSUB, NPOOL], BF16, tag="rp")
                for j in range(NSUB):
                    nc.tensor.transpose(rp[:, j, :],
                                        e_sb[:, j * SUB:(j + 1) * SUB],
                                        ident_b[:NPOOL, :NPOOL])
                att_e = attp.tile([SUB, NSUB, NPOOL], BF16, tag="att_e")
                nc.scalar.activation(att_e, rp, AF.Copy)
                sums = attp.tile([SUB, NSUB], F32, tag="sums")
                rec = attp.tile([SUB, NSUB], F32, tag="rec")
                att_n = attp.tile([SUB, NSUB, NPOOL], FP16, tag="att_n")
                nc.vector.reduce_sum(sums, att_e, axis=AX.X)
                nc.vector.reciprocal(rec, sums)
                for j in range(NSUB):
                    nc.vector.tensor_scalar_mul(
                        att_n[:, j, :], att_e[:, j, :], rec[:, j:j + 1])
                ap_ = pap.tile([NPOOL, TW], FP16, tag="ap")
                for j in range(NSUB):
                    nc.tensor.transpose(ap_[:, j * SUB:(j + 1) * SUB],
                                        att_n[:, j, :],
                                        ident_f[:SUB, :SUB])
                nc.scalar.activation(attT_store[:NPOOL, t, :], ap_,
                                     AF.Copy)

        # ================= x path: pool -> encoder -> fself -> FW =======
        finish_pool(partx, pooledx)
        with tc.tile_pool(name="ps_xe", bufs=2, space="PSUM") as pxe, \
             tc.tile_pool(name="ps_xm", bufs=1, space="PSUM") as pxm:
            encoder(pxe, pxm, pooledx, wx_sb, bex_sb, lxt_sb, bx_sb,
                    fselfT, "x")
            transpose_to_chunks(pxm, fselfT, fself_sb, "x")
            # FW = fself @ Wup'^T  [50, C]
            fwp = pxm.tile([NPOOL, C], F32, tag="fwp")
            for kc in range(KC):
                nc.tensor.matmul(fwp, fself_sb[:, kc, :], wup_sb[:, kc, :],
                                 start=(kc == 0), stop=(kc == KC - 1))
            nc.vector.tensor_copy(fw_sb[:NPOOL, :], fwp)

        # ================= out loop: out = FW^T @ attT + x ==============
        with tc.tile_pool(name="ps_o", bufs=3, space="PSUM") as pso:
            for t in range(NT):
                out_t = outp.tile([P, KC, TW], FP16, tag="out")
                for kc in range(KC):
                    op_ = pso.tile([P, TW], F32, tag="ops")
                    nc.tensor.matmul(op_,
                                     fw_sb[:, kc * P:(kc + 1) * P],
                                     attT_store[:, t, :],
                                     start=True, stop=True)
                    nc.vector.tensor_tensor(out_t[:, kc, :], op_,
                                            x_view(t, kc), ALU.add)
                nc.sync.dma_start(out=outd.ap()[:, t], in_=out_t)


def _split_lin(lw):
    # lin weight [50,50]; lhsT rows j split by pool scale -> [36, 4, 50]
    lt = lw.T.astype(np.float32)  # [j, k]
    out = np.zeros((4, S_LEN[3], NPOOL), np.float32)
    for s in range(4):
        out[s, :S_LEN[s]] = lt[S_OFF[s]:S_OFF[s] + S_LEN[s]]
    return np.ascontiguousarray(out.transpose(1, 0, 2))


def _bn_fold(bn):
    g, bt, m, v = [a.astype(np.float64) for a in bn]
    a = g / np.sqrt(v + EPS)
    return a, bt.astype(np.float64) - a * m


def _to16(a):
    return np.ascontiguousarray(a).astype(np.float16)


def _prep(inputs):
    """Host-side fold + shard. Returns list of 8 per-core input maps."""
    f = {k: np.asarray(v) for k, v in inputs.items()}

    a1, b1 = _bn_fold(f["fx_bn"][0])
    a2, b2 = _bn_fold(f["fx_bn"][1])
    W1 = f["fx_w"][0].astype(np.float64)
    W2 = f["fx_w"][1].astype(np.float64)
    Wc = (a2[:, None] * W2) @ (a1[:, None] * W1)
    bc = a2 * (W2 @ b1) + b2

    aup, bup = _bn_fold(f["fup_bn"])
    Wup = aup[:, None] * f["fup_w"].astype(np.float64)

    def enc_fold(w, bn):
        # fold BN affine AND the pool block-mean 1/n_s into the weights
        wts, bs = [], []
        for s in range(4):
            a, b = _bn_fold(bn[s])
            ws = (a[:, None] * w[s].astype(np.float64)) / S_N[s]
            wts.append(ws.T.reshape(KC, P, C))
            bs.append(b)
        # [4s, KC, P, C] -> [P, 4s*KC, C]
        wt = np.stack(wts).transpose(2, 0, 1, 3).reshape(P, 4 * KC, C)
        return _to16(wt), _to16(np.stack(bs))

    wxt, bex = enc_fold(f["enc_x_w"], f["enc_x_bn"])
    wyt, bey = enc_fold(f["enc_y_w"], f["enc_y_bn"])

    # block-indicator for PE pooling of yT: M[pix, b6] (12x12 blocks),
    # and the p6 -> [p1;p2;p3] aggregation matrix A [36, 14]
    NP_PAD = NYT * P
    pix = np.arange(NP_PAD)
    r, cc = pix // W, pix % W
    Mfull = np.zeros((NP_PAD, 36), np.float16)
    valid = pix < HW
    b6 = (r // 12) * 6 + (cc // 12)
    Mfull[valid, b6[valid]] = 1.0
    A = np.zeros((36, 14), np.float16)
    hh, wb = np.arange(36) // 6, np.arange(36) % 6
    A[:, 0] = 1.0
    A[np.arange(36), 1 + (hh // 3) * 2 + (wb // 3)] = 1.0
    A[np.arange(36), 5 + (hh // 2) * 3 + (wb // 2)] = 1.0

    common = {
        "md": np.ascontiguousarray(
            Mfull.reshape(NYT, P, 36).transpose(1, 0, 2)),
        "ad": A,
        "wxd": wxt, "wyd": wyt, "bexd": bex, "beyd": bey,
        "wcd": _to16(Wc.reshape(KC, P, C).transpose(1, 0, 2)),
        "wupd": _to16(
            np.ascontiguousarray(Wup.T).reshape(KC, P, C).transpose(1, 0, 2)),
        "bcd": _to16(bc.reshape(KC, P).T),
        "bupd": _to16(bup.reshape(1, C)),
        "lxd": _to16(_split_lin(f["lin_x_w"])),
        "lyd": _to16(_split_lin(f["lin_y_w"])),
        "bxd": f["lin_x_b"].astype(np.float32).reshape(NPOOL, 1).copy(),
        "byd": f["lin_y_b"].astype(np.float32).reshape(NPOOL, 1).copy(),
        "onesd": np.ones((1, HW), np.float16),
        "onespd": np.ones((P, 48), np.float16),
        "idfd": np.eye(P, dtype=np.float16),
        "idbd": np.eye(P).astype(ml_dtypes.bfloat16),
    }

    def shard_stream(a):
        # [C, H, W] -> [P, NCH, KC, CW] fp16
        v = a.reshape(KC, P, HW).transpose(1, 0, 2)      # [P, KC, HW]
        v = v.reshape(P, KC, NCH, CW).transpose(0, 2, 1, 3)
        return _to16(v)

    def shard_yT(a):
        # [C, H, W] -> yT padded [P, NYT, C] fp16 (pix-major, partition p
        # holds pixels p, 128+p, ... of the padded 5248-pixel image)
        v = np.zeros((NYT * P, C), np.float16)
        v[:HW] = a.reshape(C, HW).T.astype(np.float16)
        return np.ascontiguousarray(v.reshape(NYT, P, C).transpose(1, 0, 2))

    in_maps = []
    for i in range(B):
        m = dict(common)
        m["xd"] = shard_stream(f["x"][i])
        m["ytd"] = shard_yT(f["y"][i])
        in_maps.append(m)
    return in_maps


def _get_nc():
    global _NC
    if _NC is None:
        nc = bacc.Bacc("TRN2", target_bir_lowering=False)
        _NC = _emit(nc)
    return _NC


def _run(inputs, trace=False):
    nc = _get_nc()
    in_maps = _prep(inputs)
    res = run_bass_kernel_spmd(nc, in_maps, core_ids=list(range(B)),
                               trace=trace)
    out = np.empty((B, C, H, W), np.float32)
    for i in range(B):
        o = res.results[i]["outd"]                      # [P, NT, KC, TW]
        o = o.transpose(2, 0, 1, 3).reshape(C, HW)      # [C, HW]
        out[i] = o.astype(np.float32).reshape(C, H, W)
    return out, res


def kernel(**inputs) -> np.ndarray:
    out, _ = _run(inputs, trace=False)
    return out
